# revision 1
# baseline (speedup 1.0000x reference)
"""Bass/Tile kernel for nn_Attn_40424232189956 on 8 trn2 NeuronCores.

GQA attention block: q/k/v proj + rmsnorm + rope + causal attention + out proj.
B=2, T=2048, D=2048, NH=16, NKV=4, HD=128.

Sharding: tensor-parallel over heads. Each core owns 2 q-heads + the 1 kv-head
they read (q heads 2c,2c+1 -> kv head c//2), computes a full [B*T, D] partial
of the output projection; host sums the 8 partials.

Per-core kernel layout choices:
- Processed one batch at a time (halves SBUF residency of q/k/v).
- Projections feat-major: psum [feat 128, tok 512], lhsT = W^T k-tiles,
  rhs = x^T k-tiles (x transposed on host).
- RMSNorm via ones-matmul partition reduction (value 1/(128*s_h^2) folds the
  qg gain and softmax 1/sqrt(HD) into the norm factor), sqrt bias eps/s_h^2.
- Rope in hd-major reading q halves straight from PSUM (mixed partition
  windows are legal when one operand is PSUM; output window may differ).
- Attention with TRANSPOSED scores sT[kt, qt]: softmax denominator via
  ones-column matmul (partition reduction on PE), p used directly as rhs of
  the pv matmul -> no transposes of p at all. exp() without max-subtraction
  (scores bounded by sqrt(HD) after rmsnorm; fp32 exp cannot overflow).
- Causal masking: additive -30000 masks for the 4 diagonal block phases.
"""

import numpy as np

B, T, D = 2, 2048, 2048
NH, NKV = 16, 4
HD = 128
BT = B * T            # 4096
NCORES = 8
HPC = 2               # q heads per core
NKT = D // 128        # 16 contraction tiles for projections
CHUNK = 512
EPS = float(np.finfo(np.float32).eps)
MASK_NEG = -30000.0
USE_F32R = True      # float32r (1.5 cyc/row vs 2.0) for big matmuls
USE_BF16_PV = True   # bf16 p/v/ones for the attention pv + sums matmuls


def _rope_tables():
    # Matches reference.rotary_tables for T=2048 > tsl=1024 (NTK branch).
    hd = np.float32(HD)
    ar = (np.arange(0, HD, 2, dtype=np.float32) / hd).astype(np.float32)
    expo = np.power(np.float32(HD / (HD - 2.0)), ar, dtype=np.float32)
    inv = (np.float32(1.0)
           / (np.float32(10000.0)
              * np.power(np.float32(T / 1024.0), expo, dtype=np.float32)))
    f = np.outer(np.arange(T, dtype=np.float32), inv.astype(np.float32))
    return (np.cos(f).astype(np.float32).T.copy(),
            np.sin(f).astype(np.float32).T.copy())  # [64, T] hd-major


def _build_program():
    import concourse.bass as bass
    import concourse.mybir as mybir
    import concourse.tile as tile
    from concourse import bacc
    from concourse.masks import make_identity

    f32 = mybir.dt.float32
    # matmul operand dtype: float32r = TF32-like fast path on the PE
    mdt = mybir.dt.float32r if USE_F32R else f32
    nc = bacc.Bacc("TRN2", target_bir_lowering=False)

    xT = nc.dram_tensor("xT", [D, BT], mdt, kind="ExternalInput")
    qwT = nc.dram_tensor("qwT", [D, HPC * HD], mdt, kind="ExternalInput")
    kwT = nc.dram_tensor("kwT", [D, HD], mdt, kind="ExternalInput")
    vwT = nc.dram_tensor("vwT", [D, HD], mdt, kind="ExternalInput")
    owT = nc.dram_tensor("owT", [HPC * HD, D], mdt, kind="ExternalInput")
    csd = nc.dram_tensor("csd", [128, T], f32, kind="ExternalInput")
    csd2 = nc.dram_tensor("csd2", [128, T], f32, kind="ExternalInput")
    maskd = nc.dram_tensor("maskd", [128, 4, 512], f32, kind="ExternalInput")
    normod = nc.dram_tensor("normod", [128, 3, 128], mdt, kind="ExternalInput")
    normbd = nc.dram_tensor("normbd", [128, 3], f32, kind="ExternalInput")
    outd = nc.dram_tensor("o", [BT, D], f32, kind="ExternalOutput")

    with tile.TileContext(nc) as tc:
        with (
            tc.tile_pool(name="wpool", bufs=1) as wpool,
            tc.tile_pool(name="xpool", bufs=6) as xpool,
            tc.tile_pool(name="big", bufs=1) as big,
            tc.tile_pool(name="ybp", bufs=2) as ybp,
            tc.tile_pool(name="ntmp", bufs=2) as ntmp,
            tc.tile_pool(name="ntmp1", bufs=2) as ntmp1,
            tc.tile_pool(name="atmp", bufs=3) as atmp,
            tc.tile_pool(name="ppool", bufs=5) as ppool,
            tc.tile_pool(name="opool", bufs=4) as opool,
            tc.tile_pool(name="ps", bufs=6, space="PSUM") as ps,
            tc.tile_pool(name="psv", bufs=2, space="PSUM") as psv,
        ):
            # ---- resident weights / tables ----
            qw_s = wpool.tile([128, NKT, HPC * HD], mdt)
            nc.sync.dma_start(qw_s[:], qwT.rearrange("(ko p) m -> p ko m", p=128))
            kw_s = wpool.tile([128, NKT, HD], mdt)
            nc.sync.dma_start(kw_s[:], kwT.rearrange("(ko p) m -> p ko m", p=128))
            vw_s = wpool.tile([128, NKT, HD], mdt)
            nc.sync.dma_start(vw_s[:], vwT.rearrange("(ko p) m -> p ko m", p=128))
            ow_s = wpool.tile([128, HPC, D], mdt)
            nc.sync.dma_start(ow_s[:], owT.rearrange("(h p) n -> p h n", p=128))
            cs_s = wpool.tile([128, T], f32)  # rows 0:64 cos, 64:128 sin
            nc.sync.dma_start(cs_s[:], csd[:])
            cs2_s = wpool.tile([128, T], f32)  # rows 0:64 sin, 64:128 cos
            nc.sync.dma_start(cs2_s[:], csd2[:])
            mask_s = wpool.tile([128, 4, 512], f32)
            nc.sync.dma_start(mask_s[:], maskd[:])
            normo_s = wpool.tile([128, 3, 128], mdt)
            nc.sync.dma_start(normo_s[:], normod[:])
            normb_s = wpool.tile([128, 3], f32)
            nc.sync.dma_start(normb_s[:], normbd[:])
            ones_col = wpool.tile([128, 1], f32)
            nc.vector.memset(ones_col[:], 1.0)
            ones_col_bf = wpool.tile([128, 1], mybir.dt.bfloat16)
            nc.vector.memset(ones_col_bf[:], 1.0)
            ident = wpool.tile([128, 128], f32)
            make_identity(nc, ident[:])

            f32r = mybir.dt.float32r
            bf16 = mybir.dt.bfloat16

            def mmr(out, lhsT, rhs, **kw):
                nc.tensor.matmul(out, lhsT, rhs, **kw)

            sq_ = mybir.ActivationFunctionType.Square
            sqrt_ = mybir.ActivationFunctionType.Sqrt
            exp_ = mybir.ActivationFunctionType.Exp

            def norm_rope(pt, ni, dst, pos0):
                """pt: psum [128 feat, 512 tok]; ni: 0/1 q-head, 2 k;
                dst: sbuf [128, 512] slice; pos0: seq position of col 0."""
                sq = ntmp.tile([128, CHUNK], mdt, tag="sq")
                nc.scalar.activation(out=sq[:], in_=pt[:], func=sq_)
                qsb = ntmp.tile([128, CHUNK], f32, tag="qsb")
                nc.scalar.copy(out=qsb[:], in_=pt[:])
                nb = psv.tile([128, CHUNK], f32, tag="aux", name="nb")
                nc.tensor.matmul(nb[:], normo_s[:, ni, :], sq[:],
                                 start=True, stop=True)
                rs = ntmp1.tile([64, CHUNK], f32, tag="rs")
                nc.scalar.activation(out=rs[:], in_=nb[0:64, :], func=sqrt_,
                                     bias=normb_s[0:64, ni:ni + 1], scale=1.0)
                rfac = ntmp1.tile([64, CHUNK], f32, tag="rfac")
                nc.vector.reciprocal(rfac[:], rs[:])
                cs = cs_s[0:64, pos0:pos0 + CHUNK]       # cos @ base 0
                sn = cs_s[64:128, pos0:pos0 + CHUNK]     # sin @ base 64
                sn0 = cs2_s[0:64, pos0:pos0 + CHUNK]     # sin @ base 0
                cs64 = cs2_s[64:128, pos0:pos0 + CHUNK]  # cos @ base 64
                # lo-window multiplies on the idle GPSIMD engine (sbuf only)
                t1 = ntmp1.tile([64, CHUNK], f32, tag="ta")
                t2 = ntmp1.tile([64, CHUNK], f32, tag="tb")
                nc.gpsimd.tensor_mul(t1[:], qsb[0:64, :], cs)
                nc.vector.tensor_mul(t2[:], pt[64:128, :], sn)
                nc.vector.tensor_add(t1[:], t1[:], t2[:])
                nc.vector.tensor_mul(dst[0:64, :], t1[:], rfac[:])
                t3 = ntmp1.tile([64, CHUNK], f32, tag="tc")
                t4 = ntmp1.tile([64, CHUNK], f32, tag="td")
                nc.gpsimd.tensor_mul(t3[:], qsb[0:64, :], sn0)
                nc.vector.tensor_mul(t4[:], pt[64:128, :], cs64)
                nc.vector.tensor_sub(t4[:], t4[:], t3[:])
                nc.vector.tensor_mul(dst[64:128, :], t4[:], rfac[:])

            tiles = {}

            def proj_chunk(b, ci):
                if ci == 0:
                    tiles[b] = (
                        big.tile([128, HPC, T], bf16, tag="qT", name=f"qT{b}"),
                        big.tile([128, T], bf16, tag="kT", name=f"kT{b}"),
                        big.tile([128, T], bf16 if USE_BF16_PV else f32,
                                 tag="vtok", name=f"vtok{b}"),
                    )
                qT, kT, vtok = tiles[b]
                pos0 = ci * CHUNK
                t0 = b * T + pos0
                pq0 = ps.tile([128, CHUNK], f32, tag="b512", name=f"pq0_{b}_{ci}")
                pq1 = ps.tile([128, CHUNK], f32, tag="b512", name=f"pq1_{b}_{ci}")
                pk = ps.tile([128, CHUNK], f32, tag="b512", name=f"pk_{b}_{ci}")
                pv = ps.tile([128, CHUNK], f32, tag="b512", name=f"pv_{b}_{ci}")
                for ko in range(NKT):
                    xt = xpool.tile([128, CHUNK], mdt, tag="xt",
                                    name=f"xt_{b}_{ci}_{ko}")
                    nc.sync.dma_start(
                        xt[:], xT[ko * 128:(ko + 1) * 128, t0:t0 + CHUNK])
                    st = (ko == 0)
                    sp = (ko == NKT - 1)
                    mmr(pq0[:], qw_s[:, ko, 0:128], xt[:], start=st, stop=sp)
                    mmr(pq1[:], qw_s[:, ko, 128:256], xt[:], start=st, stop=sp)
                    mmr(pk[:], kw_s[:, ko, :], xt[:], start=st, stop=sp)
                    mmr(pv[:], vw_s[:, ko, :], xt[:], start=st, stop=sp)
                norm_rope(pq0, 0, qT[:, 0, pos0:pos0 + CHUNK], pos0)
                norm_rope(pq1, 1, qT[:, 1, pos0:pos0 + CHUNK], pos0)
                norm_rope(pk, 2, kT[:, pos0:pos0 + CHUNK], pos0)
                # v: psum [hd, tok] -> sbuf, then PE-transpose to token-major
                vtmp = atmp.tile([128, CHUNK], f32, tag="vtmp",
                                 name=f"vtmp_{b}_{ci}")
                nc.scalar.copy(out=vtmp[:], in_=pv[:])
                for tb in range(4):
                    vps = psv.tile([128, 512], f32, tag="aux",
                                   name=f"vps_{b}_{ci}_{tb}")[:, 0:128]
                    nc.tensor.transpose(
                        vps, vtmp[:, tb * 128:(tb + 1) * 128], ident[:])
                    dst0 = pos0 + tb * 128
                    nc.scalar.copy(out=vtok[:, dst0:dst0 + 128], in_=vps)

            def attn_oproj_group(b, g):
                qT, kT, vtok = tiles[b]
                q0 = g * 512
                kg = 4 * (g + 1)
                ybg = ybp.tile([128, HPC, 512], mdt, tag="ybg",
                               name=f"ybg_{b}_{g}")
                yts, sms = [], []
                for h in range(HPC):
                    yts.append(ps.tile([128, 512], f32, tag="b512",
                                       name=f"yt_{b}_{g}_{h}"))
                    sms.append(psv.tile([128, 512], f32, tag="aux",
                                        name=f"sm_{b}_{g}_{h}")[0:1, :])
                oc_ap = ones_col_bf[:] if USE_BF16_PV else ones_col[:]
                # interleave both heads' chains: two independent
                # stile->exp->mm pipelines keep PE and ACT saturated
                for j in range(kg):
                    k0 = j * 128
                    for h in range(HPC):
                        stile = ps.tile([128, 512], f32, tag="b512",
                                        name=f"st_{b}_{g}_{h}_{j}")
                        nc.tensor.matmul(stile[:], kT[:, k0:k0 + 128],
                                         qT[:, h, q0:q0 + 512],
                                         start=True, stop=True)
                        if j >= 4 * g:
                            nc.vector.tensor_add(
                                stile[:], stile[:], mask_s[:, j - 4 * g, :])
                        pj = ppool.tile([128, 512],
                                        bf16 if USE_BF16_PV else f32,
                                        tag="pj", name=f"pj_{b}_{g}_{h}_{j}")
                        nc.scalar.activation(out=pj[:], in_=stile[:],
                                             func=exp_)
                        nc.tensor.matmul(sms[h], oc_ap, pj[:], start=(j == 0),
                                         stop=(j == kg - 1),
                                         skip_group_check=True)
                        nc.tensor.matmul(yts[h], vtok[:, k0:k0 + 128], pj[:],
                                         start=(j == 0), stop=(j == kg - 1),
                                         skip_group_check=True)
                for h in range(HPC):
                    rrow = atmp.tile([1, 512], f32, tag="rrow",
                                     name=f"rr_{b}_{g}_{h}")
                    nc.vector.reciprocal(rrow[:], sms[h])
                    rb = atmp.tile([128, 512], f32, tag="rb",
                                   name=f"rb_{b}_{g}_{h}")
                    nc.gpsimd.partition_broadcast(rb[:], rrow[:])
                    nc.vector.tensor_mul(ybg[:, h, :], yts[h], rb[:])
                for tb in range(4):
                    row0 = b * T + q0 + tb * 128
                    for oc in range(4):
                        ops = ps.tile([128, 512], f32, tag="b512",
                                      name=f"op_{b}_{g}_{tb}_{oc}")
                        mmr(ops[:], ybg[:, 0, tb * 128:(tb + 1) * 128],
                            ow_s[:, 0, oc * 512:(oc + 1) * 512],
                            start=True, stop=False)
                        mmr(ops[:], ybg[:, 1, tb * 128:(tb + 1) * 128],
                            ow_s[:, 1, oc * 512:(oc + 1) * 512],
                            start=False, stop=True)
                        orow = opool.tile([128, 512], f32, tag="orow",
                                          name=f"or_{b}_{g}_{tb}_{oc}")
                        if oc % 2 == 0:
                            nc.vector.tensor_copy(out=orow[:], in_=ops[:])
                        else:
                            nc.scalar.copy(out=orow[:], in_=ops[:])
                        nc.sync.dma_start(
                            outd[row0:row0 + 128,
                                 oc * 512:(oc + 1) * 512], orow[:])

            for b in range(B):
                for ci in range(4):
                    proj_chunk(b, ci)
                for g in range(4):
                    attn_oproj_group(b, g)

    nc.compile()
    return nc


_CACHED = {}
LAST_EXEC_NS = None


def _run(nc, in_maps, **kwargs):
    from concourse.bass_utils import run_bass_kernel_spmd
    return run_bass_kernel_spmd(nc, in_maps, core_ids=list(range(NCORES)),
                                **kwargs)


def _make_in_maps(x, qw, kw, vw, ow, qg):
    xTf = np.ascontiguousarray(x.reshape(BT, D).T)  # [D, BT]
    cosT, sinT = _rope_tables()
    cossin = np.concatenate([cosT, sinT], axis=0)   # [128, T] cos||sin
    sincos = np.concatenate([sinT, cosT], axis=0)   # [128, T] sin||cos

    ktl = np.arange(128, dtype=np.int64)[:, None]
    qtl = np.arange(512, dtype=np.int64)[None, :]
    mask = np.zeros((128, 4, 512), np.float32)
    for r in range(4):
        mask[:, r, :] = np.where(qtl >= ktl + 128 * r, 0.0, MASK_NEG)

    in_maps = []
    for c in range(NCORES):
        h0 = HPC * c
        kvh = (h0 * NKV) // NH  # == c // 2
        qwT_c = qw[h0 * HD:(h0 + HPC) * HD, :].T.copy()
        kwT_c = kw[kvh * HD:(kvh + 1) * HD, :].T.copy()
        vwT_c = vw[kvh * HD:(kvh + 1) * HD, :].T.copy()
        owT_c = ow[:, h0 * HD:(h0 + HPC) * HD].T.copy()
        # norm constants: s_i folds qg gain and 1/sqrt(HD) attention scale
        s = np.array([qg[h0] / np.sqrt(HD), qg[h0 + 1] / np.sqrt(HD), 1.0],
                     np.float32)
        normo = np.broadcast_to(
            (1.0 / (HD * s * s))[None, :, None], (128, 3, 128)
        ).astype(np.float32).copy()
        normb = np.broadcast_to(
            (EPS / (s * s))[None, :], (128, 3)).astype(np.float32).copy()
        in_maps.append({
            "xT": xTf, "qwT": qwT_c, "kwT": kwT_c, "vwT": vwT_c,
            "owT": owT_c, "csd": cossin, "csd2": sincos, "maskd": mask,
            "normod": normo, "normbd": normb,
        })
    return in_maps


def kernel(x, qw, kw, vw, ow, qg):
    global LAST_EXEC_NS
    x = np.ascontiguousarray(x, dtype=np.float32)
    qw = np.asarray(qw, dtype=np.float32)
    kw = np.asarray(kw, dtype=np.float32)
    vw = np.asarray(vw, dtype=np.float32)
    ow = np.asarray(ow, dtype=np.float32)
    qg = np.asarray(qg, dtype=np.float32)

    if "nc" not in _CACHED:
        _CACHED["nc"] = _build_program()
    nc = _CACHED["nc"]

    in_maps = _make_in_maps(x, qw, kw, vw, ow, qg)
    res = _run(nc, in_maps)
    LAST_EXEC_NS = res.exec_time_ns
    out = res.results[0]["o"].astype(np.float64)
    for c in range(1, NCORES):
        out += res.results[c]["o"]
    return out.astype(np.float32).reshape(B, T, D)



# revision 2
# speedup vs baseline: 1.1555x; 1.1555x over previous
"""Bass/Tile kernel for nn_Attn_40424232189956 on 8 trn2 NeuronCores.

GQA attention block: q/k/v proj + rmsnorm + rope + causal attention + out proj.
B=2, T=2048, D=2048, NH=16, NKV=4, HD=128.

Sharding: 4 q-heads x 1 batch per core (core c: batch c//4, q heads
4*(c%4)..4*(c%4)+3, kv head c%4). Each (batch, kv head) pair is computed by
exactly one core -> no duplicated kv projection work. Each core emits a full
[T, D] partial of the output projection for its batch; host sums the 4
partials per batch.

Per-core kernel layout:
- Projections feat-major: psum [feat 128, tok 512], lhsT = W^T k-tiles,
  rhs = x^T k-tiles (x transposed + cast to bf16 on host). One batched DMA
  per 512-token chunk loads all 16 k-tiles.
- RMSNorm via ones-matmul partition reduction (value 1/(128*s_h^2) folds the
  qg gain and softmax 1/sqrt(HD) into the norm factor), sqrt bias eps/s_h^2.
- Rope in hd-major reading q halves straight from PSUM.
- Attention with TRANSPOSED scores sT[kt, qt]: softmax denominator via
  ones-column matmul (partition reduction on PE), p used directly as rhs of
  the pv matmul. exp() without max-subtraction (scores bounded by sqrt(HD)
  after rmsnorm). Heads processed in 2 passes of 2 (psum budget); the j-loop
  is software-pipelined: scores for step j issue before the sms/pv matmuls
  of step j-1, so the PE never waits on the exp() round trip.
- Causal masking: additive -30000 masks for the 4 diagonal block phases.
- Output written bf16 [T, D]; host sums partials in f32.
"""

import numpy as np

B, T, D = 2, 2048, 2048
NH, NKV = 16, 4
HD = 128
NCORES = 8
HPC = 4               # q heads per core
NKT = D // 128        # 16 contraction tiles for projections
CHUNK = 512
NCH = T // CHUNK      # 4 chunks
EPS = float(np.finfo(np.float32).eps)
MASK_NEG = -30000.0


def _rope_tables():
    # Matches reference.rotary_tables for T=2048 > tsl=1024 (NTK branch).
    hd = np.float32(HD)
    ar = (np.arange(0, HD, 2, dtype=np.float32) / hd).astype(np.float32)
    expo = np.power(np.float32(HD / (HD - 2.0)), ar, dtype=np.float32)
    inv = (np.float32(1.0)
           / (np.float32(10000.0)
              * np.power(np.float32(T / 1024.0), expo, dtype=np.float32)))
    f = np.outer(np.arange(T, dtype=np.float32), inv.astype(np.float32))
    return (np.cos(f).astype(np.float32).T.copy(),
            np.sin(f).astype(np.float32).T.copy())  # [64, T] hd-major


def _build_program():
    import concourse.bass as bass
    import concourse.mybir as mybir
    import concourse.tile as tile
    from concourse import bacc
    from concourse.masks import make_identity

    f32 = mybir.dt.float32
    f32r = mybir.dt.float32r
    bf16 = mybir.dt.bfloat16
    nc = bacc.Bacc("TRN2", target_bir_lowering=False)

    xT = nc.dram_tensor("xT", [D, T], bf16, kind="ExternalInput")
    qwT = nc.dram_tensor("qwT", [D, HPC * HD], bf16, kind="ExternalInput")
    kwT = nc.dram_tensor("kwT", [D, HD], bf16, kind="ExternalInput")
    vwT = nc.dram_tensor("vwT", [D, HD], bf16, kind="ExternalInput")
    owT = nc.dram_tensor("owT", [HPC * HD, D], bf16, kind="ExternalInput")
    csd = nc.dram_tensor("csd", [128, T], f32, kind="ExternalInput")
    csd2 = nc.dram_tensor("csd2", [128, T], f32, kind="ExternalInput")
    maskd = nc.dram_tensor("maskd", [128, 4, CHUNK], f32, kind="ExternalInput")
    normod = nc.dram_tensor("normod", [128, HPC + 1, 128], f32r,
                            kind="ExternalInput")
    normbd = nc.dram_tensor("normbd", [128, HPC + 1], f32, kind="ExternalInput")
    outd = nc.dram_tensor("o", [T, D], bf16, kind="ExternalOutput")

    with tile.TileContext(nc) as tc:
        with (
            tc.tile_pool(name="wpool", bufs=1) as wpool,
            tc.tile_pool(name="xpool", bufs=2) as xpool,
            tc.tile_pool(name="big", bufs=1) as big,
            tc.tile_pool(name="ybp", bufs=2) as ybp,
            tc.tile_pool(name="ntmp", bufs=2) as ntmp,
            tc.tile_pool(name="ntmp1", bufs=2) as ntmp1,
            tc.tile_pool(name="atmp", bufs=3) as atmp,
            tc.tile_pool(name="ppool", bufs=5) as ppool,
            tc.tile_pool(name="opool", bufs=2) as opool,
            tc.tile_pool(name="ps", bufs=6, space="PSUM") as ps,
            tc.tile_pool(name="psv", bufs=2, space="PSUM") as psv,
        ):
            # ---- resident weights / tables ----
            qw_s = wpool.tile([128, NKT, HPC * HD], bf16)
            nc.sync.dma_start(qw_s[:], qwT.rearrange("(ko p) m -> p ko m", p=128))
            kw_s = wpool.tile([128, NKT, HD], bf16)
            nc.sync.dma_start(kw_s[:], kwT.rearrange("(ko p) m -> p ko m", p=128))
            vw_s = wpool.tile([128, NKT, HD], bf16)
            nc.sync.dma_start(vw_s[:], vwT.rearrange("(ko p) m -> p ko m", p=128))
            ow_s = wpool.tile([128, HPC, D], bf16)
            nc.sync.dma_start(ow_s[:], owT.rearrange("(h p) n -> p h n", p=128))
            cs_s = wpool.tile([128, T], f32)  # rows 0:64 cos, 64:128 sin
            nc.sync.dma_start(cs_s[:], csd[:])
            cs2_s = wpool.tile([128, T], f32)  # rows 0:64 sin, 64:128 cos
            nc.sync.dma_start(cs2_s[:], csd2[:])
            mask_s = wpool.tile([128, 4, CHUNK], f32)
            nc.sync.dma_start(mask_s[:], maskd[:])
            normo_s = wpool.tile([128, HPC + 1, 128], f32r)
            nc.sync.dma_start(normo_s[:], normod[:])
            normb_s = wpool.tile([128, HPC + 1], f32)
            nc.sync.dma_start(normb_s[:], normbd[:])
            ones_col_bf = wpool.tile([128, 1], bf16)
            nc.vector.memset(ones_col_bf[:], 1.0)
            ident = wpool.tile([128, 128], f32)
            make_identity(nc, ident[:])

            qT = big.tile([128, HPC, T], bf16, tag="qT", name="qT")
            kT = big.tile([128, T], bf16, tag="kT", name="kT")
            vtok = big.tile([128, T], bf16, tag="vtok", name="vtok")

            sq_ = mybir.ActivationFunctionType.Square
            sqrt_ = mybir.ActivationFunctionType.Sqrt
            exp_ = mybir.ActivationFunctionType.Exp

            def norm_rope(pt, ni, dst, pos0):
                """pt: psum [128 feat, 512 tok]; ni: 0..3 q-head, 4 k;
                dst: sbuf [128, 512] slice; pos0: seq position of col 0."""
                sq = ntmp.tile([128, CHUNK], f32r, tag="sq")
                nc.scalar.activation(out=sq[:], in_=pt[:], func=sq_)
                qsb = ntmp.tile([128, CHUNK], f32, tag="qsb")
                nc.scalar.copy(out=qsb[:], in_=pt[:])
                nb = psv.tile([128, CHUNK], f32, tag="aux", name=f"nb_{ni}_{pos0}")
                nc.tensor.matmul(nb[:], normo_s[:, ni, :], sq[:],
                                 start=True, stop=True)
                rs = ntmp1.tile([64, CHUNK], f32, tag="rs")
                nc.scalar.activation(out=rs[:], in_=nb[0:64, :], func=sqrt_,
                                     bias=normb_s[0:64, ni:ni + 1], scale=1.0)
                rfac = ntmp1.tile([64, CHUNK], f32, tag="rfac")
                nc.vector.reciprocal(rfac[:], rs[:])
                cs = cs_s[0:64, pos0:pos0 + CHUNK]       # cos @ base 0
                sn = cs_s[64:128, pos0:pos0 + CHUNK]     # sin @ base 64
                sn0 = cs2_s[0:64, pos0:pos0 + CHUNK]     # sin @ base 0
                cs64 = cs2_s[64:128, pos0:pos0 + CHUNK]  # cos @ base 64
                # lo-window multiplies on the idle GPSIMD engine (sbuf only)
                t1 = ntmp1.tile([64, CHUNK], f32, tag="ta")
                t2 = ntmp1.tile([64, CHUNK], f32, tag="tb")
                nc.gpsimd.tensor_mul(t1[:], qsb[0:64, :], cs)
                nc.vector.tensor_mul(t2[:], pt[64:128, :], sn)
                nc.vector.tensor_add(t1[:], t1[:], t2[:])
                nc.vector.tensor_mul(dst[0:64, :], t1[:], rfac[:])
                t3 = ntmp1.tile([64, CHUNK], f32, tag="tc")
                t4 = ntmp1.tile([64, CHUNK], f32, tag="td")
                nc.gpsimd.tensor_mul(t3[:], qsb[0:64, :], sn0)
                nc.vector.tensor_mul(t4[:], pt[64:128, :], cs64)
                nc.vector.tensor_sub(t4[:], t4[:], t3[:])
                nc.vector.tensor_mul(dst[64:128, :], t4[:], rfac[:])

            def proj_chunk(ci):
                pos0 = ci * CHUNK
                xc = xpool.tile([128, NKT, CHUNK], bf16, tag="xc",
                                name=f"xc_{ci}")
                nc.sync.dma_start(
                    xc[:],
                    xT.rearrange("(ko p) m -> p ko m", p=128)[
                        :, :, pos0:pos0 + CHUNK])
                pq = [ps.tile([128, CHUNK], f32, tag="b512",
                              name=f"pq{h}_{ci}") for h in range(HPC)]
                pk = ps.tile([128, CHUNK], f32, tag="b512", name=f"pk_{ci}")
                pv = ps.tile([128, CHUNK], f32, tag="b512", name=f"pv_{ci}")
                for ko in range(NKT):
                    rhs = xc[:, ko, :]
                    st = (ko == 0)
                    sp = (ko == NKT - 1)
                    for h in range(HPC):
                        nc.tensor.matmul(pq[h][:],
                                         qw_s[:, ko, h * 128:(h + 1) * 128],
                                         rhs, start=st, stop=sp)
                    nc.tensor.matmul(pk[:], kw_s[:, ko, :], rhs,
                                     start=st, stop=sp)
                    nc.tensor.matmul(pv[:], vw_s[:, ko, :], rhs,
                                     start=st, stop=sp)
                for h in range(HPC):
                    norm_rope(pq[h], h, qT[:, h, pos0:pos0 + CHUNK], pos0)
                norm_rope(pk, HPC, kT[:, pos0:pos0 + CHUNK], pos0)
                # v: psum [hd, tok] -> sbuf, then PE-transpose to token-major
                vtmp = atmp.tile([128, CHUNK], f32, tag="vtmp",
                                 name=f"vtmp_{ci}")
                nc.scalar.copy(out=vtmp[:], in_=pv[:])
                for tb in range(4):
                    vps = psv.tile([128, CHUNK], f32, tag="aux",
                                   name=f"vps_{ci}_{tb}")[:, 0:128]
                    nc.tensor.transpose(
                        vps, vtmp[:, tb * 128:(tb + 1) * 128], ident[:])
                    dst0 = pos0 + tb * 128
                    nc.scalar.copy(out=vtok[:, dst0:dst0 + 128], in_=vps)

            def attn_pass(g, hh):
                """Attention for query group g, heads hh (pair). Writes
                normalized per-head outputs into ybg[:, h, :]."""
                q0 = g * CHUNK
                kg = 4 * (g + 1)
                yts, sms = {}, {}
                for h in hh:
                    yts[h] = ps.tile([128, CHUNK], f32, tag="b512",
                                     name=f"yt_{g}_{h}")
                    sms[h] = psv.tile([128, CHUNK], f32, tag="aux",
                                      name=f"sm_{g}_{h}")[0:1, :]
                pend = None  # (j, {h: pj}) awaiting sms/pv issue
                for j in range(kg):
                    k0 = j * 128
                    st = {}
                    for h in hh:  # both scores share the kT lhsT
                        st[h] = ps.tile([128, CHUNK], f32, tag="b512",
                                        name=f"st_{g}_{h}_{j}")
                        nc.tensor.matmul(st[h][:], kT[:, k0:k0 + 128],
                                         qT[:, h, q0:q0 + CHUNK],
                                         start=True, stop=True)
                    if pend is not None:
                        pj_, j_ = pend
                        for h in hh:
                            nc.tensor.matmul(sms[h], ones_col_bf[:], pj_[h][:],
                                             start=(j_ == 0), stop=False,
                                             skip_group_check=True)
                        for h in hh:
                            nc.tensor.matmul(yts[h][:],
                                             vtok[:, j_ * 128:j_ * 128 + 128],
                                             pj_[h][:],
                                             start=(j_ == 0), stop=False,
                                             skip_group_check=True)
                    pjs = {}
                    for h in hh:
                        if j >= 4 * g:
                            nc.vector.tensor_add(
                                st[h][:], st[h][:], mask_s[:, j - 4 * g, :])
                        pj = ppool.tile([128, CHUNK], bf16, tag="pj",
                                        name=f"pj_{g}_{h}_{j}")
                        nc.scalar.activation(out=pj[:], in_=st[h][:],
                                             func=exp_)
                        pjs[h] = pj
                    pend = (pjs, j)
                pj_, j_ = pend
                for h in hh:
                    nc.tensor.matmul(sms[h], ones_col_bf[:], pj_[h][:],
                                     start=(j_ == 0), stop=True,
                                     skip_group_check=True)
                for h in hh:
                    nc.tensor.matmul(yts[h][:],
                                     vtok[:, j_ * 128:j_ * 128 + 128],
                                     pj_[h][:], start=(j_ == 0), stop=True,
                                     skip_group_check=True)
                for h in hh:
                    rrow = atmp.tile([1, CHUNK], f32, tag="rrow",
                                     name=f"rr_{g}_{h}")
                    nc.vector.reciprocal(rrow[:], sms[h])
                    rb = atmp.tile([128, CHUNK], f32, tag="rb",
                                   name=f"rb_{g}_{h}")
                    nc.gpsimd.partition_broadcast(rb[:], rrow[:])
                    nc.vector.tensor_mul(ybg_cur[0][:, h, :], yts[h][:], rb[:])

            def oproj_group(g):
                q0 = g * CHUNK
                ybg = ybg_cur[0]
                for tb in range(4):
                    row0 = q0 + tb * 128
                    ops = [ps.tile([128, CHUNK], f32, tag="b512",
                                   name=f"op_{g}_{tb}_{oc}")
                           for oc in range(4)]
                    for h in range(HPC):
                        lhsT = ybg[:, h, tb * 128:(tb + 1) * 128]
                        for oc in range(4):
                            nc.tensor.matmul(
                                ops[oc][:], lhsT,
                                ow_s[:, h, oc * 512:(oc + 1) * 512],
                                start=(h == 0), stop=(h == HPC - 1),
                                skip_group_check=True)
                    orow = opool.tile([128, D], bf16, tag="orow",
                                      name=f"or_{g}_{tb}")
                    for oc in range(4):
                        dst = orow[:, oc * 512:(oc + 1) * 512]
                        if oc % 2 == 0:
                            nc.vector.tensor_copy(out=dst, in_=ops[oc][:])
                        else:
                            nc.scalar.copy(out=dst, in_=ops[oc][:])
                    nc.sync.dma_start(outd[row0:row0 + 128, :], orow[:])

            ybg_cur = [None]
            for c in range(NCH):
                proj_chunk(c)
                ybg_cur[0] = ybp.tile([128, HPC, CHUNK], bf16, tag="ybg",
                                      name=f"ybg_{c}")
                attn_pass(c, (0, 1))
                attn_pass(c, (2, 3))
                oproj_group(c)

    nc.compile()
    return nc


_CACHED = {}
LAST_EXEC_NS = None


def _run(nc, in_maps, **kwargs):
    from concourse.bass_utils import run_bass_kernel_spmd
    return run_bass_kernel_spmd(nc, in_maps, core_ids=list(range(NCORES)),
                                **kwargs)


def _make_in_maps(x, qw, kw, vw, ow, qg):
    import ml_dtypes
    bf = ml_dtypes.bfloat16
    cosT, sinT = _rope_tables()
    cossin = np.concatenate([cosT, sinT], axis=0)   # [128, T] cos||sin
    sincos = np.concatenate([sinT, cosT], axis=0)   # [128, T] sin||cos

    ktl = np.arange(128, dtype=np.int64)[:, None]
    qtl = np.arange(CHUNK, dtype=np.int64)[None, :]
    mask = np.zeros((128, 4, CHUNK), np.float32)
    for r in range(4):
        mask[:, r, :] = np.where(qtl >= ktl + 128 * r, 0.0, MASK_NEG)

    xTb = [np.ascontiguousarray(x[b].T).astype(bf) for b in range(B)]

    in_maps = []
    for c in range(NCORES):
        bi, hg = divmod(c, HPC)
        h0 = HPC * hg
        qwT_c = qw[h0 * HD:(h0 + HPC) * HD, :].T.astype(bf).copy()
        kwT_c = kw[hg * HD:(hg + 1) * HD, :].T.astype(bf).copy()
        vwT_c = vw[hg * HD:(hg + 1) * HD, :].T.astype(bf).copy()
        owT_c = ow[:, h0 * HD:(h0 + HPC) * HD].T.astype(bf).copy()
        # norm constants: s_i folds qg gain and 1/sqrt(HD) attention scale
        s = np.array([qg[h0 + i] / np.sqrt(HD) for i in range(HPC)] + [1.0],
                     np.float32)
        normo = np.broadcast_to(
            (1.0 / (HD * s * s))[None, :, None], (128, HPC + 1, 128)
        ).astype(np.float32).copy()
        normb = np.broadcast_to(
            (EPS / (s * s))[None, :], (128, HPC + 1)).astype(np.float32).copy()
        in_maps.append({
            "xT": xTb[bi], "qwT": qwT_c, "kwT": kwT_c, "vwT": vwT_c,
            "owT": owT_c, "csd": cossin, "csd2": sincos, "maskd": mask,
            "normod": normo, "normbd": normb,
        })
    return in_maps


def kernel(x, qw, kw, vw, ow, qg):
    global LAST_EXEC_NS
    x = np.ascontiguousarray(x, dtype=np.float32)
    qw = np.asarray(qw, dtype=np.float32)
    kw = np.asarray(kw, dtype=np.float32)
    vw = np.asarray(vw, dtype=np.float32)
    ow = np.asarray(ow, dtype=np.float32)
    qg = np.asarray(qg, dtype=np.float32)

    if "nc" not in _CACHED:
        _CACHED["nc"] = _build_program()
    nc = _CACHED["nc"]

    in_maps = _make_in_maps(x, qw, kw, vw, ow, qg)
    res = _run(nc, in_maps)
    LAST_EXEC_NS = res.exec_time_ns
    out = np.zeros((B, T, D), np.float64)
    for c in range(NCORES):
        bi = c // HPC
        out[bi] += res.results[c]["o"].astype(np.float64)
    return out.astype(np.float32)
